# revision 51
# baseline (speedup 1.0000x reference)
"""Multi-head attention with RoPE (B=32, N=577, C=768, H=12, D=64) on 8 TRN2 NeuronCores.

Strategy: data-parallel over batch (4 images per core), zero collectives.
Per-core layout: channels-on-partitions, tokens-on-free-dim throughout.
  - w_qkv rows permuted on host so each head's q,k land as [32 even dims;
    32 odd dims] contiguously per head (head pair per 128-row tile). RoPE
    is then rot = A*C4 + swap32(A)*S4 where swap32 is four 32-row
    SBUF-to-SBUF DMAs; C4/S4 built on host (col 0 = identity for CLS).
  - scores_T[j, i] per head via one K=64 matmul (heads at row strips 0-1 /
    2-3 run concurrently); softmax over the partition dim without
    max-subtraction (scores*scale ~ N(0,1)); exp on ScalarE fused with
    the 1/sqrt(d) scale, output bf16.
  - v computed in [token, channel] layout with a ones-column per head so
    attn@v (M=65) also yields the softmax denominator in row 64.
  - denominators: batched per image, 1/s via ln+exp(-x) on ScalarE,
    partition-broadcast via a one-hot selector matmul on the PE,
    normalization on VectorE.
  - two-level software pipeline: next image's QKV/V matmul units are
    interleaved into the current image's attention pair loop; each
    image's projection is deferred one image to hide the denominator
    chain. All matmuls bf16 with fp32 PSUM accumulation.
Output computed as [b, c, t] on device; host transposes back.
"""

import sys

sys.path.insert(0, "/opt/trn_rl_repo")

import numpy as np
import ml_dtypes

import concourse.bass as bass
import concourse.bacc as bacc
import concourse.tile as tile
from concourse import mybir
from concourse.bass_utils import run_bass_kernel_spmd

F32 = mybir.dt.float32
BF16 = mybir.dt.bfloat16

B, N, C = 32, 577, 768
H, D = 12, 64
NCORES = 8
BL = B // NCORES  # images per core
SCALE = D ** -0.5
NT = 5  # token tiles: 4*128 + 65
TWS = [128, 128, 128, 128, 65]
# free-dim chunks (psum-bank aligned)
NCH = [(0, 512), (512, 65)]
VCH = [(0, 512), (512, 256)]


def build(n_images=BL):
    nc = bacc.Bacc()
    xT = nc.declare_dram_parameter("xT", [n_images, C, N], BF16, isOutput=False)
    wqk = nc.declare_dram_parameter("wqk", [C, 2 * C], BF16, isOutput=False)
    wv = nc.declare_dram_parameter("wv", [C, C], BF16, isOutput=False)
    wp = nc.declare_dram_parameter("wp", [C, C], BF16, isOutput=False)
    c4d = nc.declare_dram_parameter("c4", [128, N], BF16, isOutput=False)
    s4d = nc.declare_dram_parameter("s4", [128, N], BF16, isOutput=False)
    bpd = nc.declare_dram_parameter("bproj", [6, 128], F32, isOutput=False)
    seld = nc.declare_dram_parameter("sel", [12, 12 * 64], BF16, isOutput=False)
    out = nc.declare_dram_parameter("out", [n_images, C, N], F32, isOutput=True)

    Exp = mybir.ActivationFunctionType.Exp
    MUL = mybir.AluOpType.mult
    ADD = mybir.AluOpType.add

    with tile.TileContext(nc) as tc:
        with (
            tc.tile_pool(name="wpool", bufs=1) as wpool,
            tc.tile_pool(name="xp", bufs=2) as xp,
            tc.tile_pool(name="qkp", bufs=2) as qkp,
            tc.tile_pool(name="vp", bufs=2) as vp,
            tc.tile_pool(name="ep", bufs=3) as ep,
            tc.tile_pool(name="ap", bufs=2) as app,
            tc.tile_pool(name="tp", bufs=3) as tp,
            tc.tile_pool(name="rp", bufs=2) as rp,
            tc.tile_pool(name="op", bufs=3) as op_,
            tc.tile_pool(name="ps", bufs=4, space="PSUM") as ps,
            tc.tile_pool(name="dp", bufs=4, space="DRAM") as dp,
        ):
            # ---- qkv-critical loads first; PE warm-up burst hides the DMA head ----
            wqk_sb = []
            wv_sb = []
            wp_sb = []
            for k in range(6):
                t = wpool.tile([128, 2 * C], BF16, tag=f"wqk{k}", name=f"wqk{k}")
                nc.sync.dma_start(out=t[:], in_=wqk[k * 128:(k + 1) * 128, :])
                wqk_sb.append(t)
            c4 = wpool.tile([128, N], BF16, tag="c4")
            nc.sync.dma_start(out=c4[:], in_=c4d[:])
            s4 = wpool.tile([128, N], BF16, tag="s4")
            nc.sync.dma_start(out=s4[:], in_=s4d[:])
            # HAM warm-up: ~20 dummy matmuls on a memset tile while input DMAs stream
            wu = wpool.tile([128, 512], BF16, tag="wu")
            nc.vector.memset(wu[:], 0.5)
            wups = ps.tile([128, 512], F32, tag="ps", name="wups", bufs=4)
            for _ in range(34):
                nc.tensor.matmul(out=wups[:, 0:512], lhsT=wu[:, 0:128], rhs=wu[:, 0:512],
                                 start=True, stop=True)

            def load_wv():
                for k in range(6):
                    t = wpool.tile([128, C], BF16, tag=f"wv{k}", name=f"wv{k}")
                    nc.sync.dma_start(out=t[:], in_=wv[k * 128:(k + 1) * 128, :])
                    wv_sb.append(t)

            def load_rest():
                for k in range(6):
                    t = wpool.tile([128, C], BF16, tag=f"wp{k}", name=f"wp{k}")
                    nc.sync.dma_start(out=t[:], in_=wp[k * 128:(k + 1) * 128, :])
                    wp_sb.append(t)
                t = wpool.tile([128, 6], F32, tag="b")
                nc.sync.dma_start(out=t[:], in_=bpd[:].transpose([1, 0]))
                wp_sb.append(t)
                t = wpool.tile([12, 12 * 64], BF16, tag="sel")
                nc.sync.dma_start(out=t[:], in_=seld[:])
                wp_sb.append(t)

            pending_proj = []

            def emit_proj_ct(pb, pattn, ct):
                    pp = [ps.tile([128, 512], F32, tag="ps", name="pp0", bufs=4),
                          ps.tile([128, 512], F32, tag="ps", name="pp1", bufs=4)]
                    for k in range(6):
                        lhsT = wp_sb[k][:, ct * 128:(ct + 1) * 128]
                        for ci, (c0, cw) in enumerate(NCH):
                            nc.tensor.matmul(
                                out=pp[ci][:, 0:cw],
                                lhsT=lhsT,
                                rhs=pattn[:, k, c0:c0 + cw],
                                start=(k == 0),
                                stop=(k == 5),
                            )
                    osb = op_.tile([128, N], F32, tag="osb")
                    nc.vector.tensor_scalar_add(out=osb[:, 0:512], in0=pp[0][:, 0:512], scalar1=bsb[:, ct:ct + 1])
                    nc.vector.tensor_scalar_add(out=osb[:, 512:N], in0=pp[1][:, 0:65], scalar1=bsb[:, ct:ct + 1])
                    nc.sync.dma_start(out=out[pb, ct * 128:(ct + 1) * 128, :], in_=osb[:])

            def emit_proj(pb, pattn):
                for ct in range(6):
                    emit_proj_ct(pb, pattn, ct)

            def emit_x_loads(b):
                xsb = []
                for k in range(6):
                    t = xp.tile([128, N], BF16, tag=f"x{k}", name=f"x{k}")
                    nc.sync.dma_start(out=t[:], in_=xT[b, k * 128:(k + 1) * 128, :])
                    xsb.append(t)
                return xsb

            def emit_qkv_unit(xsb, qk_all, m):
                pq = [ps.tile([128, 512], F32, tag="ps", name="pq0", bufs=4),
                      ps.tile([128, 512], F32, tag="ps", name="pq1", bufs=4)]
                lhs_col = m * 128
                for k in range(6):
                    lhsT = wqk_sb[k][:, lhs_col:lhs_col + 128]
                    for ci, (c0, cw) in enumerate(NCH):
                        nc.tensor.matmul(
                            out=pq[ci][:, 0:cw],
                            lhsT=lhsT,
                            rhs=xsb[k][:, c0:c0 + cw],
                            start=(k == 0),
                            stop=(k == 5),
                        )
                # RoPE: rot = A*C4 + pairswap(A)*S4   (col 0: c=1, s=0)
                raw = tp.tile([128, N], BF16, tag="roperaw", name="raw")
                nc.vector.tensor_copy(out=raw[:, 0:512], in_=pq[0][:, 0:512])
                nc.vector.tensor_copy(out=raw[:, 512:N], in_=pq[1][:, 0:65])
                sw = tp.tile([128, N], BF16, tag="ropesw", name="sw")
                nc.gpsimd.dma_start(out=sw[0:32, :], in_=raw[32:64, :])
                nc.sync.dma_start(out=sw[32:64, :], in_=raw[0:32, :])
                nc.gpsimd.dma_start(out=sw[64:96, :], in_=raw[96:128, :])
                nc.sync.dma_start(out=sw[96:128, :], in_=raw[64:96, :])
                tmp = tp.tile([128, N], BF16, tag="ropetmp", name="tmp")
                rot = tp.tile([128, N], BF16, tag="roperot", name="rot")
                nc.vector.tensor_tensor(out=tmp[:], in0=sw[:], in1=s4[:], op=MUL)
                nc.vector.tensor_tensor(out=rot[:], in0=raw[:], in1=c4[:], op=MUL)
                nc.vector.tensor_tensor(out=qk_all[:, m, :], in0=rot[:], in1=tmp[:], op=ADD)

            def emit_v_unit(xsb, v_all, t_i):
                tw = TWS[t_i]
                t0 = t_i * 128
                pv = [ps.tile([128, 512], F32, tag="ps", name="pv0", bufs=4),
                      ps.tile([128, 512], F32, tag="ps", name="pv1", bufs=4)]
                for k in range(6):
                    lhsT = xsb[k][:, t0:t0 + tw]
                    for ci, (c0, cw) in enumerate(VCH):
                        nc.tensor.matmul(
                            out=pv[ci][0:tw, 0:cw],
                            lhsT=lhsT,
                            rhs=wv_sb[k][:, c0:c0 + cw],
                            start=(k == 0),
                            stop=(k == 5),
                        )
                vdst = v_all[0:tw, t_i, :].rearrange("p (h c) -> p h c", c=65)
                nc.vector.tensor_copy(
                    out=vdst[:, 0:8, 0:64],
                    in_=pv[0][0:tw, :].rearrange("p (h d) -> p h d", d=64),
                )
                nc.vector.tensor_copy(
                    out=vdst[:, 8:12, 0:64],
                    in_=pv[1][0:tw, 0:256].rearrange("p (h d) -> p h d", d=64),
                )
                nc.vector.memset(vdst[:, 0:12, 64], 1.0)

            def emit_scores(qk_all, all_exps, m):
                qt = qk_all[:, m, :]
                kt = qk_all[:, 6 + m, :]
                exps = [
                    ep.tile([128, NT, N], BF16, tag="expA", name="expA"),
                    ep.tile([128, NT, N], BF16, tag="expB", name="expB"),
                ]
                all_exps[m] = exps
                for j in range(NT):
                    jw = TWS[j]
                    j0 = j * 128
                    pscs = [ps.tile([128, 1024], F32, tag="ps2", name="pscA", bufs=2),
                            ps.tile([128, 1024], F32, tag="ps2", name="pscB", bufs=2)]
                    for c0, cw in NCH:
                        for hh in range(2):
                            r0, r1_ = hh * 64, hh * 64 + 64
                            nc.tensor.matmul(out=pscs[hh][0:jw, c0:c0 + cw], lhsT=kt[r0:r1_, j0:j0 + jw],
                                             rhs=qt[r0:r1_, c0:c0 + cw], start=True, stop=True)
                    for hh in range(2):
                        nc.scalar.activation(out=exps[hh][0:jw, j, :], in_=pscs[hh][0:jw, 0:N],
                                             func=Exp, scale=SCALE)

            def emit_attnv(v_all, all_exps, araws, sums12, m):
                a65s = []
                araws.append(a65s)
                exps = all_exps.pop(m)
                for hh in range(2):
                    h = 2 * m + hh
                    po = [ps.tile([128, 512], F32, tag="ps", name="po0", bufs=4),
                          ps.tile([128, 512], F32, tag="ps", name="po1", bufs=4)]
                    for j in range(NT):
                        jw = TWS[j]
                        lhsT = v_all[0:jw, j, :].rearrange("p (h c) -> p h c", c=65)[:, h, :]
                        for ci, (c0, cw) in enumerate(NCH):
                            nc.tensor.matmul(
                                out=po[ci][0:65, 0:cw],
                                lhsT=lhsT,
                                rhs=exps[hh][0:jw, j, c0:c0 + cw],
                                start=(j == 0),
                                stop=(j == NT - 1),
                            )
                    # free psum fast: pull out attn rows + sums row (bf16), gather sums via DMA
                    a65 = rp.tile([65, N], BF16, tag="a65", name="a65", bufs=14)
                    a65s.append(a65)
                    nc.vector.tensor_copy(out=a65[:, 0:512], in_=po[0][0:65, 0:512])
                    nc.vector.tensor_copy(out=a65[:, 512:N], in_=po[1][0:65, 0:65])
                    nc.gpsimd.dma_start(out=sums12[h:h + 1, :], in_=a65[64:65, :])

            # ---- software-pipelined image loop ----
            xsb0 = emit_x_loads(0)
            load_wv()
            qk0 = qkp.tile([128, 12, N], BF16, tag="qk", name="qk0")
            v0 = vp.tile([128, NT, 13 * 65], BF16, tag="v", name="v0")
            for m in range(4):
                emit_qkv_unit(xsb0, qk0, m)
            load_rest()
            bsb = wp_sb[6]
            sel = wp_sb[7]
            del wp_sb[6:]
            for m in range(4, 12):
                emit_qkv_unit(xsb0, qk0, m)
            for t_i in range(NT):
                emit_v_unit(xsb0, v0, t_i)
            cur = (qk0, v0)

            for b in range(n_images):
                qk_all, v_all = cur
                next_units = []
                if b + 1 < n_images:
                    xsbn = emit_x_loads(b + 1)
                    qkn = qkp.tile([128, 12, N], BF16, tag="qk", name="qkn")
                    vn = vp.tile([128, NT, 13 * 65], BF16, tag="v", name="vn")
                    next_units = [(lambda mm: (lambda: emit_qkv_unit(xsbn, qkn, mm)))(mm) for mm in range(12)] + \
                                 [(lambda tt: (lambda: emit_v_unit(xsbn, vn, tt)))(tt) for tt in range(NT)]
                    cur = (qkn, vn)
                else:
                    # last image: interleave the previous image's projection as PE filler
                    for pb_, pattn_ in pending_proj:
                        next_units += [(lambda c, p1, p2: (lambda: emit_proj_ct(p1, p2, c)))(c, pb_, pattn_)
                                       for c in range(6)]
                    pending_proj = []

                attn_all = app.tile([128, 6, N], BF16, tag="attn")
                sums12 = rp.tile([12, N], BF16, tag="sums12", bufs=2)
                araws = []
                all_exps = {}
                nu = len(next_units)
                take_per_step = [(nu * (m + 1)) // 7 - (nu * m) // 7 for m in range(7)]
                ui = 0
                for m in range(7):
                    if m < 6:
                        emit_scores(qk_all, all_exps, m)
                    if m >= 1:
                        emit_attnv(v_all, all_exps, araws, sums12, m - 1)
                    for _ in range(take_per_step[m]):
                        if ui < len(next_units):
                            next_units[ui]()
                            ui += 1
                while ui < len(next_units):
                    next_units[ui]()
                    ui += 1

                # batch reciprocal of all 12 denominators, PE-broadcast (bf16 sel matmul),
                # normalize pairs interleaved with the previous image's projection tiles
                # so PE density stays high across the image boundary
                r12 = rp.tile([12, N], BF16, tag="r12", bufs=2)
                lns = rp.tile([12, N], F32, tag="lns", bufs=2)
                nc.scalar.activation(out=lns[:], in_=sums12[:], func=mybir.ActivationFunctionType.Ln)
                nc.scalar.activation(out=r12[:], in_=lns[:], func=Exp, scale=-1.0)
                for m in range(6):
                    if pending_proj:
                        emit_proj_ct(pending_proj[0][0], pending_proj[0][1], m)
                    rb = [ps.tile([128, 512], F32, tag="ps", name="rb0", bufs=4),
                          ps.tile([128, 512], F32, tag="ps", name="rb1", bufs=4)]
                    for ci, (c0, cw) in enumerate(NCH):
                        nc.tensor.matmul(
                            out=rb[ci][:, 0:cw],
                            lhsT=sel[:, 2 * m * 64:(2 * m + 2) * 64],
                            rhs=r12[:, c0:c0 + cw],
                            start=True, stop=True,
                        )
                        for hh in range(2):
                            nc.vector.tensor_tensor(
                                out=attn_all[hh * 64:(hh + 1) * 64, m, c0:c0 + cw],
                                in0=araws[m][hh][0:64, c0:c0 + cw],
                                in1=rb[ci][hh * 64:(hh + 1) * 64, 0:cw], op=MUL)
                pending_proj = []

                pending_proj.append((b, attn_all))
            for pb, pattn in pending_proj:
                emit_proj(pb, pattn)
    nc.compile()
    return nc


def _qk_perm():
    """Row permutation of w_qkv's q,k sections -> head-interleaved pair-split."""
    perm = np.zeros(2 * C, dtype=np.int64)
    for m in range(12):
        sec = 0 if m < 6 else 1
        pair = m % 6
        base = m * 128
        hA, hB = 2 * pair, 2 * pair + 1
        perm[base + 0:base + 32] = sec * C + hA * D + 2 * np.arange(32)
        perm[base + 32:base + 64] = sec * C + hA * D + 2 * np.arange(32) + 1
        perm[base + 64:base + 96] = sec * C + hB * D + 2 * np.arange(32)
        perm[base + 96:base + 128] = sec * C + hB * D + 2 * np.arange(32) + 1
    return perm


def prep_inputs(x, w_qkv, w_proj, b_proj, cos, sin, n_images=BL):
    bf16 = ml_dtypes.bfloat16
    perm = _qk_perm()
    wqk = np.ascontiguousarray(w_qkv[perm, :].T).astype(bf16)  # [C, 2C]
    wv = np.ascontiguousarray(w_qkv[2 * C:3 * C, :].T).astype(bf16)  # [C, C]
    wp = np.ascontiguousarray(w_proj.T).astype(bf16)  # [C(in), C(out)]

    c4 = np.ones((128, N), dtype=np.float32)
    s4 = np.zeros((128, N), dtype=np.float32)
    p = np.arange(128)
    c4[:, 1:] = cos[:, p % 32].T
    s4[:, 1:] = sin[:, p % 32].T * np.where((p // 32) % 2 == 0, -1.0, 1.0)[:, None]
    c4 = c4.astype(bf16)
    s4 = s4.astype(bf16)

    bp = np.ascontiguousarray(b_proj.reshape(6, 128)).astype(np.float32)
    selm = np.zeros((12, 12 * 64), dtype=bf16)
    for h in range(12):
        selm[h, h * 64:(h + 1) * 64] = 1.0

    xT = np.ascontiguousarray(np.transpose(x, (0, 2, 1))).astype(bf16)  # [B, C, N]

    in_maps = []
    for i in range(NCORES):
        in_maps.append({
            "xT": xT[i * n_images:(i + 1) * n_images],
            "wqk": wqk, "wv": wv, "wp": wp,
            "c4": c4, "s4": s4, "bproj": bp, "sel": selm,
        })
    return in_maps


_BUILT = {}


def kernel(x, w_qkv, w_proj, b_proj, cos, sin):
    x = np.asarray(x, dtype=np.float32)
    w_qkv = np.asarray(w_qkv, dtype=np.float32)
    w_proj = np.asarray(w_proj, dtype=np.float32)
    b_proj = np.asarray(b_proj, dtype=np.float32)
    cos = np.asarray(cos, dtype=np.float32)
    sin = np.asarray(sin, dtype=np.float32)

    if "nc" not in _BUILT:
        _BUILT["nc"] = build()
    nc = _BUILT["nc"]
    in_maps = prep_inputs(x, w_qkv, w_proj, b_proj, cos, sin)
    res = run_bass_kernel_spmd(nc, in_maps, core_ids=list(range(NCORES)))
    outs = np.concatenate([np.asarray(res.results[i]["out"]) for i in range(NCORES)], axis=0)
    return np.ascontiguousarray(np.transpose(outs, (0, 2, 1))).astype(np.float32)


# revision 52
# speedup vs baseline: 1.1558x; 1.1558x over previous
"""Multi-head attention with RoPE (B=32, N=577, C=768, H=12, D=64) on 8 TRN2 NeuronCores.

Strategy: data-parallel over batch (4 images per core), zero collectives.
Per-core layout: channels-on-partitions, tokens-on-free-dim throughout.
  - w_qkv rows permuted on host so each head's q,k land as [32 even dims;
    32 odd dims] contiguously per head (head pair per 128-row tile). RoPE
    is then rot = A*C4 + swap32(A)*S4 where swap32 is four 32-row
    SBUF-to-SBUF DMAs; C4/S4 built on host (col 0 = identity for CLS).
  - scores_T[j, i] per head via one K=64 matmul (heads at row strips 0-1 /
    2-3 run concurrently); softmax over the partition dim without
    max-subtraction (scores*scale ~ N(0,1)); exp on ScalarE fused with
    the 1/sqrt(d) scale, output bf16.
  - v computed in [token, channel] layout with a ones-column per head so
    attn@v (M=65) also yields the softmax denominator in row 64.
  - denominators: batched per image, 1/s via ln+exp(-x) on ScalarE,
    partition-broadcast via a one-hot selector matmul on the PE,
    normalization on VectorE.
  - two-level software pipeline: next image's QKV/V matmul units are
    interleaved into the current image's attention pair loop; each
    image's projection is deferred one image to hide the denominator
    chain. All matmuls bf16 with fp32 PSUM accumulation.
Output computed as [b, c, t] on device; host transposes back.
"""

import sys

sys.path.insert(0, "/opt/trn_rl_repo")

import numpy as np
import ml_dtypes

import concourse.bass as bass
import concourse.bacc as bacc
import concourse.tile as tile
from concourse import mybir
from concourse.bass_utils import run_bass_kernel_spmd

F32 = mybir.dt.float32
BF16 = mybir.dt.bfloat16

B, N, C = 32, 577, 768
H, D = 12, 64
NCORES = 8
BL = B // NCORES  # images per core
SCALE = D ** -0.5
NT = 5  # token tiles: 4*128 + 65
TWS = [128, 128, 128, 128, 65]
# free-dim chunks (psum-bank aligned)
NCH = [(0, 512), (512, 65)]
VCH = [(0, 512), (512, 256)]


def build(n_images=BL):
    nc = bacc.Bacc()
    xT = nc.declare_dram_parameter("xT", [n_images, C, N], BF16, isOutput=False)
    wqk = nc.declare_dram_parameter("wqk", [C, 2 * C], BF16, isOutput=False)
    wv = nc.declare_dram_parameter("wv", [C, C], BF16, isOutput=False)
    wp = nc.declare_dram_parameter("wp", [C, C], BF16, isOutput=False)
    c4d = nc.declare_dram_parameter("c4", [128, N], BF16, isOutput=False)
    s4d = nc.declare_dram_parameter("s4", [128, N], BF16, isOutput=False)
    bpd = nc.declare_dram_parameter("bproj", [6, 128], F32, isOutput=False)
    seld = nc.declare_dram_parameter("sel", [12, 12 * 64], BF16, isOutput=False)
    out = nc.declare_dram_parameter("out", [n_images, C, N], F32, isOutput=True)

    Exp = mybir.ActivationFunctionType.Exp
    MUL = mybir.AluOpType.mult
    ADD = mybir.AluOpType.add

    with tile.TileContext(nc) as tc:
        with (
            tc.tile_pool(name="wpool", bufs=1) as wpool,
            tc.tile_pool(name="xp", bufs=2) as xp,
            tc.tile_pool(name="qkp", bufs=2) as qkp,
            tc.tile_pool(name="vp", bufs=2) as vp,
            tc.tile_pool(name="ep", bufs=3) as ep,
            tc.tile_pool(name="ap", bufs=2) as app,
            tc.tile_pool(name="tp", bufs=3) as tp,
            tc.tile_pool(name="rp", bufs=2) as rp,
            tc.tile_pool(name="op", bufs=3) as op_,
            tc.tile_pool(name="ps", bufs=4, space="PSUM") as ps,
            tc.tile_pool(name="dp", bufs=4, space="DRAM") as dp,
        ):
            # ---- qkv-critical loads first; PE warm-up burst hides the DMA head ----
            wqk_sb = []
            wv_sb = []
            wp_sb = []
            for k in range(6):
                t = wpool.tile([128, 2 * C], BF16, tag=f"wqk{k}", name=f"wqk{k}")
                nc.sync.dma_start(out=t[:], in_=wqk[k * 128:(k + 1) * 128, :])
                wqk_sb.append(t)
            c4 = wpool.tile([128, N], BF16, tag="c4")
            nc.sync.dma_start(out=c4[:], in_=c4d[:])
            s4 = wpool.tile([128, N], BF16, tag="s4")
            nc.sync.dma_start(out=s4[:], in_=s4d[:])
            # HAM warm-up: ~20 dummy matmuls on a memset tile while input DMAs stream
            wu = wpool.tile([128, 512], BF16, tag="wu")
            nc.vector.memset(wu[:], 0.5)
            wups = ps.tile([128, 512], F32, tag="ps", name="wups", bufs=4)
            for _ in range(34):
                nc.tensor.matmul(out=wups[:, 0:512], lhsT=wu[:, 0:128], rhs=wu[:, 0:512],
                                 start=True, stop=True)

            def load_wv():
                for k in range(6):
                    t = wpool.tile([128, C], BF16, tag=f"wv{k}", name=f"wv{k}")
                    nc.sync.dma_start(out=t[:], in_=wv[k * 128:(k + 1) * 128, :])
                    wv_sb.append(t)

            def load_rest():
                for k in range(6):
                    t = wpool.tile([128, C], BF16, tag=f"wp{k}", name=f"wp{k}")
                    nc.sync.dma_start(out=t[:], in_=wp[k * 128:(k + 1) * 128, :])
                    wp_sb.append(t)
                t = wpool.tile([128, 6], F32, tag="b")
                nc.sync.dma_start(out=t[:], in_=bpd[:].transpose([1, 0]))
                wp_sb.append(t)
                t = wpool.tile([12, 12 * 64], BF16, tag="sel")
                nc.sync.dma_start(out=t[:], in_=seld[:])
                wp_sb.append(t)

            pending_proj = []

            def emit_proj_ct(pb, pattn, ct):
                    pp = [ps.tile([128, 512], F32, tag="ps", name="pp0", bufs=4),
                          ps.tile([128, 512], F32, tag="ps", name="pp1", bufs=4)]
                    for k in range(6):
                        lhsT = wp_sb[k][:, ct * 128:(ct + 1) * 128]
                        for ci, (c0, cw) in enumerate(NCH):
                            nc.tensor.matmul(
                                out=pp[ci][:, 0:cw],
                                lhsT=lhsT,
                                rhs=pattn[:, k, c0:c0 + cw],
                                start=(k == 0),
                                stop=(k == 5),
                            )
                    osb = op_.tile([128, N], F32, tag="osb")
                    nc.vector.tensor_scalar_add(out=osb[:, 0:512], in0=pp[0][:, 0:512], scalar1=bsb[:, ct:ct + 1])
                    nc.vector.tensor_scalar_add(out=osb[:, 512:N], in0=pp[1][:, 0:65], scalar1=bsb[:, ct:ct + 1])
                    nc.sync.dma_start(out=out[pb, ct * 128:(ct + 1) * 128, :], in_=osb[:])

            def emit_proj(pb, pattn):
                for ct in range(6):
                    emit_proj_ct(pb, pattn, ct)

            def emit_x_loads(b):
                xsb = []
                for k in range(6):
                    t = xp.tile([128, N], BF16, tag=f"x{k}", name=f"x{k}")
                    nc.sync.dma_start(out=t[:], in_=xT[b, k * 128:(k + 1) * 128, :])
                    xsb.append(t)
                return xsb

            def emit_qkv_unit(xsb, qk_all, m):
                pq = [ps.tile([128, 512], F32, tag="ps", name="pq0", bufs=4),
                      ps.tile([128, 512], F32, tag="ps", name="pq1", bufs=4)]
                lhs_col = m * 128
                for k in range(6):
                    lhsT = wqk_sb[k][:, lhs_col:lhs_col + 128]
                    for ci, (c0, cw) in enumerate(NCH):
                        nc.tensor.matmul(
                            out=pq[ci][:, 0:cw],
                            lhsT=lhsT,
                            rhs=xsb[k][:, c0:c0 + cw],
                            start=(k == 0),
                            stop=(k == 5),
                        )
                # RoPE: rot = A*C4 + pairswap(A)*S4   (col 0: c=1, s=0)
                raw = tp.tile([128, N], BF16, tag="roperaw", name="raw")
                nc.vector.tensor_copy(out=raw[:, 0:512], in_=pq[0][:, 0:512])
                nc.vector.tensor_copy(out=raw[:, 512:N], in_=pq[1][:, 0:65])
                sw = tp.tile([128, N], BF16, tag="ropesw", name="sw")
                nc.gpsimd.dma_start(out=sw[0:32, :], in_=raw[32:64, :])
                nc.sync.dma_start(out=sw[32:64, :], in_=raw[0:32, :])
                nc.gpsimd.dma_start(out=sw[64:96, :], in_=raw[96:128, :])
                nc.sync.dma_start(out=sw[96:128, :], in_=raw[64:96, :])
                tmp = tp.tile([128, N], BF16, tag="ropetmp", name="tmp")
                rot = tp.tile([128, N], BF16, tag="roperot", name="rot")
                nc.vector.tensor_tensor(out=tmp[:], in0=sw[:], in1=s4[:], op=MUL)
                nc.vector.tensor_tensor(out=rot[:], in0=raw[:], in1=c4[:], op=MUL)
                nc.vector.tensor_tensor(out=qk_all[:, m, :], in0=rot[:], in1=tmp[:], op=ADD)

            def emit_v_unit(xsb, v_all, t_i):
                tw = TWS[t_i]
                t0 = t_i * 128
                pv = [ps.tile([128, 512], F32, tag="ps", name="pv0", bufs=4),
                      ps.tile([128, 512], F32, tag="ps", name="pv1", bufs=4)]
                for k in range(6):
                    lhsT = xsb[k][:, t0:t0 + tw]
                    for ci, (c0, cw) in enumerate(VCH):
                        nc.tensor.matmul(
                            out=pv[ci][0:tw, 0:cw],
                            lhsT=lhsT,
                            rhs=wv_sb[k][:, c0:c0 + cw],
                            start=(k == 0),
                            stop=(k == 5),
                        )
                vdst = v_all[0:tw, t_i, :].rearrange("p (h c) -> p h c", c=65)
                nc.vector.tensor_copy(
                    out=vdst[:, 0:8, 0:64],
                    in_=pv[0][0:tw, :].rearrange("p (h d) -> p h d", d=64),
                )
                nc.vector.tensor_copy(
                    out=vdst[:, 8:12, 0:64],
                    in_=pv[1][0:tw, 0:256].rearrange("p (h d) -> p h d", d=64),
                )
                nc.vector.memset(vdst[:, 0:12, 64], 1.0)

            def emit_scores(qk_all, all_exps, m):
                qt = qk_all[:, m, :]
                kt = qk_all[:, 6 + m, :]
                exps = [
                    ep.tile([128, NT, N], BF16, tag="expA", name="expA"),
                    ep.tile([128, NT, N], BF16, tag="expB", name="expB"),
                ]
                all_exps[m] = exps
                for j in range(NT):
                    jw = TWS[j]
                    j0 = j * 128
                    pscs = [ps.tile([128, 1024], F32, tag="ps2", name="pscA", bufs=2),
                            ps.tile([128, 1024], F32, tag="ps2", name="pscB", bufs=2)]
                    for c0, cw in NCH:
                        for hh in range(2):
                            r0, r1_ = hh * 64, hh * 64 + 64
                            nc.tensor.matmul(out=pscs[hh][0:jw, c0:c0 + cw], lhsT=kt[r0:r1_, j0:j0 + jw],
                                             rhs=qt[r0:r1_, c0:c0 + cw], start=True, stop=True)
                    for hh in range(2):
                        nc.scalar.activation(out=exps[hh][0:jw, j, :], in_=pscs[hh][0:jw, 0:N],
                                             func=Exp, scale=SCALE)

            def emit_attnv(v_all, all_exps, araws, sums12, m):
                a65s = []
                araws.append(a65s)
                exps = all_exps.pop(m)
                for hh in range(2):
                    h = 2 * m + hh
                    po = [ps.tile([128, 512], F32, tag="ps", name="po0", bufs=4),
                          ps.tile([128, 512], F32, tag="ps", name="po1", bufs=4)]
                    for j in range(NT):
                        jw = TWS[j]
                        lhsT = v_all[0:jw, j, :].rearrange("p (h c) -> p h c", c=65)[:, h, :]
                        for ci, (c0, cw) in enumerate(NCH):
                            nc.tensor.matmul(
                                out=po[ci][0:65, 0:cw],
                                lhsT=lhsT,
                                rhs=exps[hh][0:jw, j, c0:c0 + cw],
                                start=(j == 0),
                                stop=(j == NT - 1),
                            )
                    # free psum fast: pull out attn rows + sums row (bf16), gather sums via DMA
                    a65 = rp.tile([65, N], BF16, tag="a65", name="a65", bufs=14)
                    a65s.append(a65)
                    nc.vector.tensor_copy(out=a65[:, 0:512], in_=po[0][0:65, 0:512])
                    nc.vector.tensor_copy(out=a65[:, 512:N], in_=po[1][0:65, 0:65])
                    nc.gpsimd.dma_start(out=sums12[h:h + 1, :], in_=a65[64:65, :])

            # ---- software-pipelined image loop ----
            xsb0 = emit_x_loads(0)
            load_wv()
            qk0 = qkp.tile([128, 12, N], BF16, tag="qk", name="qk0")
            v0 = vp.tile([128, NT, 13 * 65], BF16, tag="v", name="v0")
            for m in range(4):
                emit_qkv_unit(xsb0, qk0, m)
            load_rest()
            bsb = wp_sb[6]
            sel = wp_sb[7]
            del wp_sb[6:]
            for m in range(4, 12):
                emit_qkv_unit(xsb0, qk0, m)
            for t_i in range(NT):
                emit_v_unit(xsb0, v0, t_i)
            cur = (qk0, v0)

            for b in range(n_images):
                qk_all, v_all = cur
                next_units = []
                if b + 1 < n_images:
                    xsbn = emit_x_loads(b + 1)
                    qkn = qkp.tile([128, 12, N], BF16, tag="qk", name="qkn")
                    vn = vp.tile([128, NT, 13 * 65], BF16, tag="v", name="vn")
                    next_units = [(lambda mm: (lambda: emit_qkv_unit(xsbn, qkn, mm)))(mm) for mm in range(12)] + \
                                 [(lambda tt: (lambda: emit_v_unit(xsbn, vn, tt)))(tt) for tt in range(NT)]
                    cur = (qkn, vn)
                else:
                    # last image: interleave the previous image's projection as PE filler
                    for pb_, pattn_ in pending_proj:
                        next_units += [(lambda c, p1, p2: (lambda: emit_proj_ct(p1, p2, c)))(c, pb_, pattn_)
                                       for c in range(6)]
                    pending_proj = []

                attn_all = app.tile([128, 6, N], BF16, tag="attn")
                sums12 = rp.tile([12, N], BF16, tag="sums12", bufs=2)
                araws = []
                all_exps = {}
                take_per_step = [3, 3, 3, 2, 2, 2, 2]
                ui = 0
                for m in range(7):
                    if m < 6:
                        emit_scores(qk_all, all_exps, m)
                    if m >= 1:
                        emit_attnv(v_all, all_exps, araws, sums12, m - 1)
                    for _ in range(take_per_step[m]):
                        if ui < len(next_units):
                            next_units[ui]()
                            ui += 1
                while ui < len(next_units):
                    next_units[ui]()
                    ui += 1

                # batch reciprocal of all 12 denominators, PE-broadcast (bf16 sel matmul),
                # normalize pairs interleaved with the previous image's projection tiles
                # so PE density stays high across the image boundary
                r12 = rp.tile([12, N], BF16, tag="r12", bufs=2)
                lns = rp.tile([12, N], F32, tag="lns", bufs=2)
                nc.scalar.activation(out=lns[:], in_=sums12[:], func=mybir.ActivationFunctionType.Ln)
                nc.scalar.activation(out=r12[:], in_=lns[:], func=Exp, scale=-1.0)
                for m in range(6):
                    if pending_proj:
                        emit_proj_ct(pending_proj[0][0], pending_proj[0][1], m)
                    rb = [ps.tile([128, 512], F32, tag="ps", name="rb0", bufs=4),
                          ps.tile([128, 512], F32, tag="ps", name="rb1", bufs=4)]
                    for ci, (c0, cw) in enumerate(NCH):
                        nc.tensor.matmul(
                            out=rb[ci][:, 0:cw],
                            lhsT=sel[:, 2 * m * 64:(2 * m + 2) * 64],
                            rhs=r12[:, c0:c0 + cw],
                            start=True, stop=True,
                        )
                        for hh in range(2):
                            nc.vector.tensor_tensor(
                                out=attn_all[hh * 64:(hh + 1) * 64, m, c0:c0 + cw],
                                in0=araws[m][hh][0:64, c0:c0 + cw],
                                in1=rb[ci][hh * 64:(hh + 1) * 64, 0:cw], op=MUL)
                pending_proj = []

                pending_proj.append((b, attn_all))
            for pb, pattn in pending_proj:
                emit_proj(pb, pattn)
    nc.compile()
    return nc


def _qk_perm():
    """Row permutation of w_qkv's q,k sections -> head-interleaved pair-split."""
    perm = np.zeros(2 * C, dtype=np.int64)
    for m in range(12):
        sec = 0 if m < 6 else 1
        pair = m % 6
        base = m * 128
        hA, hB = 2 * pair, 2 * pair + 1
        perm[base + 0:base + 32] = sec * C + hA * D + 2 * np.arange(32)
        perm[base + 32:base + 64] = sec * C + hA * D + 2 * np.arange(32) + 1
        perm[base + 64:base + 96] = sec * C + hB * D + 2 * np.arange(32)
        perm[base + 96:base + 128] = sec * C + hB * D + 2 * np.arange(32) + 1
    return perm


def prep_inputs(x, w_qkv, w_proj, b_proj, cos, sin, n_images=BL):
    bf16 = ml_dtypes.bfloat16
    perm = _qk_perm()
    wqk = np.ascontiguousarray(w_qkv[perm, :].T).astype(bf16)  # [C, 2C]
    wv = np.ascontiguousarray(w_qkv[2 * C:3 * C, :].T).astype(bf16)  # [C, C]
    wp = np.ascontiguousarray(w_proj.T).astype(bf16)  # [C(in), C(out)]

    c4 = np.ones((128, N), dtype=np.float32)
    s4 = np.zeros((128, N), dtype=np.float32)
    p = np.arange(128)
    c4[:, 1:] = cos[:, p % 32].T
    s4[:, 1:] = sin[:, p % 32].T * np.where((p // 32) % 2 == 0, -1.0, 1.0)[:, None]
    c4 = c4.astype(bf16)
    s4 = s4.astype(bf16)

    bp = np.ascontiguousarray(b_proj.reshape(6, 128)).astype(np.float32)
    selm = np.zeros((12, 12 * 64), dtype=bf16)
    for h in range(12):
        selm[h, h * 64:(h + 1) * 64] = 1.0

    xT = np.ascontiguousarray(np.transpose(x, (0, 2, 1))).astype(bf16)  # [B, C, N]

    in_maps = []
    for i in range(NCORES):
        in_maps.append({
            "xT": xT[i * n_images:(i + 1) * n_images],
            "wqk": wqk, "wv": wv, "wp": wp,
            "c4": c4, "s4": s4, "bproj": bp, "sel": selm,
        })
    return in_maps


_BUILT = {}


def kernel(x, w_qkv, w_proj, b_proj, cos, sin):
    x = np.asarray(x, dtype=np.float32)
    w_qkv = np.asarray(w_qkv, dtype=np.float32)
    w_proj = np.asarray(w_proj, dtype=np.float32)
    b_proj = np.asarray(b_proj, dtype=np.float32)
    cos = np.asarray(cos, dtype=np.float32)
    sin = np.asarray(sin, dtype=np.float32)

    if "nc" not in _BUILT:
        _BUILT["nc"] = build()
    nc = _BUILT["nc"]
    in_maps = prep_inputs(x, w_qkv, w_proj, b_proj, cos, sin)
    res = run_bass_kernel_spmd(nc, in_maps, core_ids=list(range(NCORES)))
    outs = np.concatenate([np.asarray(res.results[i]["out"]) for i in range(NCORES)], axis=0)
    return np.ascontiguousarray(np.transpose(outs, (0, 2, 1))).astype(np.float32)


# revision 53
# speedup vs baseline: 1.1674x; 1.0101x over previous
"""Multi-head attention with RoPE (B=32, N=577, C=768, H=12, D=64) on 8 TRN2 NeuronCores.

Strategy: data-parallel over batch (4 images per core), zero collectives.
Per-core layout: channels-on-partitions, tokens-on-free-dim throughout.
  - w_qkv rows permuted on host so each head's q,k land as [32 even dims;
    32 odd dims] contiguously per head (head pair per 128-row tile). RoPE
    is then rot = A*C4 + swap32(A)*S4 where swap32 is four 32-row
    SBUF-to-SBUF DMAs; C4/S4 built on host (col 0 = identity for CLS).
  - scores_T[j, i] per head via one K=64 matmul (heads at row strips 0-1 /
    2-3 run concurrently); softmax over the partition dim without
    max-subtraction (scores*scale ~ N(0,1)); exp on ScalarE fused with
    the 1/sqrt(d) scale, output bf16.
  - v computed in [token, channel] layout with a ones-column per head so
    attn@v (M=65) also yields the softmax denominator in row 64.
  - denominators: batched per image, 1/s via ln+exp(-x) on ScalarE,
    partition-broadcast via a one-hot selector matmul on the PE,
    normalization on VectorE.
  - two-level software pipeline: next image's QKV/V matmul units are
    interleaved into the current image's attention pair loop; each
    image's projection is deferred one image to hide the denominator
    chain. All matmuls bf16 with fp32 PSUM accumulation.
Output computed as [b, c, t] on device; host transposes back.
"""

import sys

sys.path.insert(0, "/opt/trn_rl_repo")

import numpy as np
import ml_dtypes

import concourse.bass as bass
import concourse.bacc as bacc
import concourse.tile as tile
from concourse import mybir
from concourse.bass_utils import run_bass_kernel_spmd

F32 = mybir.dt.float32
BF16 = mybir.dt.bfloat16

B, N, C = 32, 577, 768
H, D = 12, 64
NCORES = 8
BL = B // NCORES  # images per core
SCALE = D ** -0.5
NT = 5  # token tiles: 4*128 + 65
TWS = [128, 128, 128, 128, 65]
# free-dim chunks (psum-bank aligned)
NCH = [(0, 512), (512, 65)]
VCH = [(0, 512), (512, 256)]


def build(n_images=BL):
    nc = bacc.Bacc()
    xT = nc.declare_dram_parameter("xT", [n_images, C, N], BF16, isOutput=False)
    wqk = nc.declare_dram_parameter("wqk", [C, 2 * C], BF16, isOutput=False)
    wv = nc.declare_dram_parameter("wv", [C, C], BF16, isOutput=False)
    wp = nc.declare_dram_parameter("wp", [C, C], BF16, isOutput=False)
    c4d = nc.declare_dram_parameter("c4", [128, N], BF16, isOutput=False)
    s4d = nc.declare_dram_parameter("s4", [128, N], BF16, isOutput=False)
    bpd = nc.declare_dram_parameter("bproj", [6, 128], F32, isOutput=False)
    seld = nc.declare_dram_parameter("sel", [12, 12 * 64], BF16, isOutput=False)
    out = nc.declare_dram_parameter("out", [n_images, C, N], F32, isOutput=True)

    Exp = mybir.ActivationFunctionType.Exp
    MUL = mybir.AluOpType.mult
    ADD = mybir.AluOpType.add

    with tile.TileContext(nc) as tc:
        with (
            tc.tile_pool(name="wpool", bufs=1) as wpool,
            tc.tile_pool(name="xp", bufs=2) as xp,
            tc.tile_pool(name="qkp", bufs=2) as qkp,
            tc.tile_pool(name="vp", bufs=2) as vp,
            tc.tile_pool(name="ep", bufs=3) as ep,
            tc.tile_pool(name="ap", bufs=2) as app,
            tc.tile_pool(name="tp", bufs=3) as tp,
            tc.tile_pool(name="rp", bufs=2) as rp,
            tc.tile_pool(name="op", bufs=3) as op_,
            tc.tile_pool(name="ps", bufs=4, space="PSUM") as ps,
            tc.tile_pool(name="dp", bufs=4, space="DRAM") as dp,
        ):
            # ---- qkv-critical loads first; PE warm-up burst hides the DMA head ----
            wqk_sb = []
            wv_sb = []
            wp_sb = []
            for k in range(6):
                t = wpool.tile([128, 2 * C], BF16, tag=f"wqk{k}", name=f"wqk{k}")
                nc.sync.dma_start(out=t[:], in_=wqk[k * 128:(k + 1) * 128, :])
                wqk_sb.append(t)
            c4 = wpool.tile([128, N], BF16, tag="c4")
            nc.sync.dma_start(out=c4[:], in_=c4d[:])
            s4 = wpool.tile([128, N], BF16, tag="s4")
            nc.sync.dma_start(out=s4[:], in_=s4d[:])
            # HAM warm-up: ~20 dummy matmuls on a memset tile while input DMAs stream
            wu = wpool.tile([128, 512], BF16, tag="wu")
            nc.vector.memset(wu[:], 0.5)
            wups = ps.tile([128, 512], F32, tag="ps", name="wups", bufs=4)
            for _ in range(34):
                nc.tensor.matmul(out=wups[:, 0:512], lhsT=wu[:, 0:128], rhs=wu[:, 0:512],
                                 start=True, stop=True)

            def load_wv():
                for k in range(6):
                    t = wpool.tile([128, C], BF16, tag=f"wv{k}", name=f"wv{k}")
                    nc.sync.dma_start(out=t[:], in_=wv[k * 128:(k + 1) * 128, :])
                    wv_sb.append(t)

            def load_rest():
                for k in range(6):
                    t = wpool.tile([128, C], BF16, tag=f"wp{k}", name=f"wp{k}")
                    nc.sync.dma_start(out=t[:], in_=wp[k * 128:(k + 1) * 128, :])
                    wp_sb.append(t)
                t = wpool.tile([128, 6], F32, tag="b")
                nc.sync.dma_start(out=t[:], in_=bpd[:].transpose([1, 0]))
                wp_sb.append(t)
                t = wpool.tile([12, 12 * 64], BF16, tag="sel")
                nc.sync.dma_start(out=t[:], in_=seld[:])
                wp_sb.append(t)

            pending_proj = []

            def emit_proj_ct(pb, pattn, ct):
                    pp = [ps.tile([128, 512], F32, tag="ps", name="pp0", bufs=4),
                          ps.tile([128, 512], F32, tag="ps", name="pp1", bufs=4)]
                    for k in range(6):
                        lhsT = wp_sb[k][:, ct * 128:(ct + 1) * 128]
                        for ci, (c0, cw) in enumerate(NCH):
                            nc.tensor.matmul(
                                out=pp[ci][:, 0:cw],
                                lhsT=lhsT,
                                rhs=pattn[:, k, c0:c0 + cw],
                                start=(k == 0),
                                stop=(k == 5),
                            )
                    osb = op_.tile([128, N], F32, tag="osb")
                    nc.vector.tensor_scalar_add(out=osb[:, 0:512], in0=pp[0][:, 0:512], scalar1=bsb[:, ct:ct + 1])
                    nc.vector.tensor_scalar_add(out=osb[:, 512:N], in0=pp[1][:, 0:65], scalar1=bsb[:, ct:ct + 1])
                    nc.sync.dma_start(out=out[pb, ct * 128:(ct + 1) * 128, :], in_=osb[:])

            def emit_proj(pb, pattn):
                for ct in range(6):
                    emit_proj_ct(pb, pattn, ct)

            def emit_x_loads(b):
                xsb = []
                for k in range(6):
                    t = xp.tile([128, N], BF16, tag=f"x{k}", name=f"x{k}")
                    nc.sync.dma_start(out=t[:], in_=xT[b, k * 128:(k + 1) * 128, :])
                    xsb.append(t)
                return xsb

            def emit_qkv_unit(xsb, qk_all, m):
                pq = [ps.tile([128, 512], F32, tag="ps", name="pq0", bufs=4),
                      ps.tile([128, 512], F32, tag="ps", name="pq1", bufs=4)]
                lhs_col = m * 128
                for k in range(6):
                    lhsT = wqk_sb[k][:, lhs_col:lhs_col + 128]
                    for ci, (c0, cw) in enumerate(NCH):
                        nc.tensor.matmul(
                            out=pq[ci][:, 0:cw],
                            lhsT=lhsT,
                            rhs=xsb[k][:, c0:c0 + cw],
                            start=(k == 0),
                            stop=(k == 5),
                        )
                # RoPE: rot = A*C4 + pairswap(A)*S4   (col 0: c=1, s=0)
                raw = tp.tile([128, N], BF16, tag="roperaw", name="raw")
                nc.vector.tensor_copy(out=raw[:, 0:512], in_=pq[0][:, 0:512])
                nc.vector.tensor_copy(out=raw[:, 512:N], in_=pq[1][:, 0:65])
                sw = tp.tile([128, N], BF16, tag="ropesw", name="sw")
                nc.gpsimd.dma_start(out=sw[0:32, :], in_=raw[32:64, :])
                nc.sync.dma_start(out=sw[32:64, :], in_=raw[0:32, :])
                nc.gpsimd.dma_start(out=sw[64:96, :], in_=raw[96:128, :])
                nc.sync.dma_start(out=sw[96:128, :], in_=raw[64:96, :])
                tmp = tp.tile([128, N], BF16, tag="ropetmp", name="tmp")
                rot = tp.tile([128, N], BF16, tag="roperot", name="rot")
                nc.vector.tensor_tensor(out=tmp[:], in0=sw[:], in1=s4[:], op=MUL)
                nc.vector.tensor_tensor(out=rot[:], in0=raw[:], in1=c4[:], op=MUL)
                nc.vector.tensor_tensor(out=qk_all[:, m, :], in0=rot[:], in1=tmp[:], op=ADD)

            def emit_v_unit(xsb, v_all, t_i):
                tw = TWS[t_i]
                t0 = t_i * 128
                pv = [ps.tile([128, 512], F32, tag="ps", name="pv0", bufs=4),
                      ps.tile([128, 512], F32, tag="ps", name="pv1", bufs=4)]
                for k in range(6):
                    lhsT = xsb[k][:, t0:t0 + tw]
                    for ci, (c0, cw) in enumerate(VCH):
                        nc.tensor.matmul(
                            out=pv[ci][0:tw, 0:cw],
                            lhsT=lhsT,
                            rhs=wv_sb[k][:, c0:c0 + cw],
                            start=(k == 0),
                            stop=(k == 5),
                        )
                vdst = v_all[0:tw, t_i, :].rearrange("p (h c) -> p h c", c=65)
                nc.vector.tensor_copy(
                    out=vdst[:, 0:8, 0:64],
                    in_=pv[0][0:tw, :].rearrange("p (h d) -> p h d", d=64),
                )
                nc.vector.tensor_copy(
                    out=vdst[:, 8:12, 0:64],
                    in_=pv[1][0:tw, 0:256].rearrange("p (h d) -> p h d", d=64),
                )
                nc.vector.memset(vdst[:, 0:12, 64], 1.0)

            def emit_scores(qk_all, all_exps, m):
                qt = qk_all[:, m, :]
                kt = qk_all[:, 6 + m, :]
                exps = [
                    ep.tile([128, NT, N], BF16, tag="expA", name="expA"),
                    ep.tile([128, NT, N], BF16, tag="expB", name="expB"),
                ]
                all_exps[m] = exps
                for j in range(NT):
                    jw = TWS[j]
                    j0 = j * 128
                    pscs = [ps.tile([128, 1024], F32, tag="ps2", name="pscA", bufs=2),
                            ps.tile([128, 1024], F32, tag="ps2", name="pscB", bufs=2)]
                    for c0, cw in NCH:
                        for hh in range(2):
                            r0, r1_ = hh * 64, hh * 64 + 64
                            nc.tensor.matmul(out=pscs[hh][0:jw, c0:c0 + cw], lhsT=kt[r0:r1_, j0:j0 + jw],
                                             rhs=qt[r0:r1_, c0:c0 + cw], start=True, stop=True)
                    for hh in range(2):
                        nc.scalar.activation(out=exps[hh][0:jw, j, :], in_=pscs[hh][0:jw, 0:N],
                                             func=Exp, scale=SCALE)

            def emit_attnv(v_all, all_exps, araws, sums12, m):
                a65s = []
                araws.append(a65s)
                exps = all_exps.pop(m)
                for hh in range(2):
                    h = 2 * m + hh
                    po = [ps.tile([128, 512], F32, tag="ps", name="po0", bufs=4),
                          ps.tile([128, 512], F32, tag="ps", name="po1", bufs=4)]
                    for j in range(NT):
                        jw = TWS[j]
                        lhsT = v_all[0:jw, j, :].rearrange("p (h c) -> p h c", c=65)[:, h, :]
                        for ci, (c0, cw) in enumerate(NCH):
                            nc.tensor.matmul(
                                out=po[ci][0:65, 0:cw],
                                lhsT=lhsT,
                                rhs=exps[hh][0:jw, j, c0:c0 + cw],
                                start=(j == 0),
                                stop=(j == NT - 1),
                            )
                    # free psum fast: pull out attn rows + sums row (bf16), gather sums via DMA
                    a65 = rp.tile([65, N], BF16, tag="a65", name="a65", bufs=14)
                    a65s.append(a65)
                    nc.vector.tensor_copy(out=a65[:, 0:512], in_=po[0][0:65, 0:512])
                    nc.vector.tensor_copy(out=a65[:, 512:N], in_=po[1][0:65, 0:65])
                    nc.gpsimd.dma_start(out=sums12[h:h + 1, :], in_=a65[64:65, :])

            # ---- software-pipelined image loop ----
            xsb0 = emit_x_loads(0)
            load_wv()
            qk0 = qkp.tile([128, 12, N], BF16, tag="qk", name="qk0")
            v0 = vp.tile([128, NT, 13 * 65], BF16, tag="v", name="v0")
            for m in range(4):
                emit_qkv_unit(xsb0, qk0, m)
            load_rest()
            bsb = wp_sb[6]
            sel = wp_sb[7]
            del wp_sb[6:]
            for m in range(4, 12):
                emit_qkv_unit(xsb0, qk0, m)
            for t_i in range(NT):
                emit_v_unit(xsb0, v0, t_i)
            cur = (qk0, v0)

            for b in range(n_images):
                qk_all, v_all = cur
                next_units = []
                if b + 1 < n_images:
                    xsbn = emit_x_loads(b + 1)
                    qkn = qkp.tile([128, 12, N], BF16, tag="qk", name="qkn")
                    vn = vp.tile([128, NT, 13 * 65], BF16, tag="v", name="vn")
                    next_units = [(lambda mm: (lambda: emit_qkv_unit(xsbn, qkn, mm)))(mm) for mm in range(12)] + \
                                 [(lambda tt: (lambda: emit_v_unit(xsbn, vn, tt)))(tt) for tt in range(NT)]
                    cur = (qkn, vn)
                else:
                    # last image: interleave the previous image's projection as PE filler
                    for pb_, pattn_ in pending_proj:
                        next_units += [(lambda c, p1, p2: (lambda: emit_proj_ct(p1, p2, c)))(c, pb_, pattn_)
                                       for c in range(6)]
                    pending_proj = []

                attn_all = app.tile([128, 6, N], BF16, tag="attn")
                sums12 = rp.tile([12, N], BF16, tag="sums12", bufs=2)
                araws = []
                all_exps = {}
                if b + 1 < n_images:
                    take_per_step = [3, 3, 3, 2, 2, 2, 2]
                else:
                    take_per_step = [1, 1, 1, 1, 1, 1, 0]
                ui = 0
                for m in range(7):
                    if m < 6:
                        emit_scores(qk_all, all_exps, m)
                    if m >= 1:
                        emit_attnv(v_all, all_exps, araws, sums12, m - 1)
                    for _ in range(take_per_step[m]):
                        if ui < len(next_units):
                            next_units[ui]()
                            ui += 1
                while ui < len(next_units):
                    next_units[ui]()
                    ui += 1

                # batch reciprocal of all 12 denominators, PE-broadcast (bf16 sel matmul),
                # normalize pairs interleaved with the previous image's projection tiles
                # so PE density stays high across the image boundary
                r12 = rp.tile([12, N], BF16, tag="r12", bufs=2)
                lns = rp.tile([12, N], F32, tag="lns", bufs=2)
                nc.scalar.activation(out=lns[:], in_=sums12[:], func=mybir.ActivationFunctionType.Ln)
                nc.scalar.activation(out=r12[:], in_=lns[:], func=Exp, scale=-1.0)
                for m in range(6):
                    if pending_proj:
                        emit_proj_ct(pending_proj[0][0], pending_proj[0][1], m)
                    rb = [ps.tile([128, 512], F32, tag="ps", name="rb0", bufs=4),
                          ps.tile([128, 512], F32, tag="ps", name="rb1", bufs=4)]
                    for ci, (c0, cw) in enumerate(NCH):
                        nc.tensor.matmul(
                            out=rb[ci][:, 0:cw],
                            lhsT=sel[:, 2 * m * 64:(2 * m + 2) * 64],
                            rhs=r12[:, c0:c0 + cw],
                            start=True, stop=True,
                        )
                        for hh in range(2):
                            nc.vector.tensor_tensor(
                                out=attn_all[hh * 64:(hh + 1) * 64, m, c0:c0 + cw],
                                in0=araws[m][hh][0:64, c0:c0 + cw],
                                in1=rb[ci][hh * 64:(hh + 1) * 64, 0:cw], op=MUL)
                pending_proj = []

                pending_proj.append((b, attn_all))
            for pb, pattn in pending_proj:
                emit_proj(pb, pattn)
    nc.compile()
    return nc


def _qk_perm():
    """Row permutation of w_qkv's q,k sections -> head-interleaved pair-split."""
    perm = np.zeros(2 * C, dtype=np.int64)
    for m in range(12):
        sec = 0 if m < 6 else 1
        pair = m % 6
        base = m * 128
        hA, hB = 2 * pair, 2 * pair + 1
        perm[base + 0:base + 32] = sec * C + hA * D + 2 * np.arange(32)
        perm[base + 32:base + 64] = sec * C + hA * D + 2 * np.arange(32) + 1
        perm[base + 64:base + 96] = sec * C + hB * D + 2 * np.arange(32)
        perm[base + 96:base + 128] = sec * C + hB * D + 2 * np.arange(32) + 1
    return perm


def prep_inputs(x, w_qkv, w_proj, b_proj, cos, sin, n_images=BL):
    bf16 = ml_dtypes.bfloat16
    perm = _qk_perm()
    wqk = np.ascontiguousarray(w_qkv[perm, :].T).astype(bf16)  # [C, 2C]
    wv = np.ascontiguousarray(w_qkv[2 * C:3 * C, :].T).astype(bf16)  # [C, C]
    wp = np.ascontiguousarray(w_proj.T).astype(bf16)  # [C(in), C(out)]

    c4 = np.ones((128, N), dtype=np.float32)
    s4 = np.zeros((128, N), dtype=np.float32)
    p = np.arange(128)
    c4[:, 1:] = cos[:, p % 32].T
    s4[:, 1:] = sin[:, p % 32].T * np.where((p // 32) % 2 == 0, -1.0, 1.0)[:, None]
    c4 = c4.astype(bf16)
    s4 = s4.astype(bf16)

    bp = np.ascontiguousarray(b_proj.reshape(6, 128)).astype(np.float32)
    selm = np.zeros((12, 12 * 64), dtype=bf16)
    for h in range(12):
        selm[h, h * 64:(h + 1) * 64] = 1.0

    xT = np.ascontiguousarray(np.transpose(x, (0, 2, 1))).astype(bf16)  # [B, C, N]

    in_maps = []
    for i in range(NCORES):
        in_maps.append({
            "xT": xT[i * n_images:(i + 1) * n_images],
            "wqk": wqk, "wv": wv, "wp": wp,
            "c4": c4, "s4": s4, "bproj": bp, "sel": selm,
        })
    return in_maps


_BUILT = {}


def kernel(x, w_qkv, w_proj, b_proj, cos, sin):
    x = np.asarray(x, dtype=np.float32)
    w_qkv = np.asarray(w_qkv, dtype=np.float32)
    w_proj = np.asarray(w_proj, dtype=np.float32)
    b_proj = np.asarray(b_proj, dtype=np.float32)
    cos = np.asarray(cos, dtype=np.float32)
    sin = np.asarray(sin, dtype=np.float32)

    if "nc" not in _BUILT:
        _BUILT["nc"] = build()
    nc = _BUILT["nc"]
    in_maps = prep_inputs(x, w_qkv, w_proj, b_proj, cos, sin)
    res = run_bass_kernel_spmd(nc, in_maps, core_ids=list(range(NCORES)))
    outs = np.concatenate([np.asarray(res.results[i]["out"]) for i in range(NCORES)], axis=0)
    return np.ascontiguousarray(np.transpose(outs, (0, 2, 1))).astype(np.float32)
